# revision 46
# baseline (speedup 1.0000x reference)
"""Bass/Tile Trainium2 kernel for masked-bank BatchConv2D.

Math (matches the reference nn.Module):
    mask[o, j]   = j < connect_nums[o]                       (j in [0, 64))
    kdense[o, c] = sum_{j : j%32==c} weights[o, j] * mask[o, j]   -> [64, 32, 3, 3]
    out          = conv2d(x, kdense, VALID) + bias[None]          -> [B, 64, 126, 126]

Strategy: data-parallel over batch (8 cores x 4 images). Per core, conv is
computed as 3 accumulating bf16 matmuls (one per kernel-column dx) with the
contraction dim packed as (dy, c) = 96 partitions; psum output partitions
are column-pair-tiled as rows (y, y+64), so one [128, 504] psum tile holds
4 output rows from each image half.

Default mode "v2" (~105-110 us, rel err ~3.7e-3 vs the 2e-2 gate):
  - whole-image x tile [96, H*W] bf16; the raw image lands in partitions
    0:32 via f32->bf16 cast DMAs (8 row-chunks in group-consumption order),
    and the dy=1 / dy=2 row-shifted partition blocks are built by ACT / DVE
    copies so they stay off the DMA engines (the bottleneck resource).
  - the (y, y+64) pairing lets bias load from HBM exactly once (lower rows
    to partitions 0:64, upper to 64:128 at the same free offset) and keeps
    the psum+bias evacuation a single [128, N] vector add per pair.
  - output is stored as bf16 (halves store traffic; the host upcasts after
    gather) in 16-contiguous-row blocks.
  - scheduling: weight prep is 5 DVE ops on host-transposed weights (no PE
    transpose); x loads are prefetched 1.5 images ahead (xi pool holds 3);
    shift copies are issued 4 groups ahead on ACT but only 2 on DVE so they
    never head-of-line-block the adds; bias is staggered one group ahead of
    its consumers. The kernel is limited by DMA engine throughput under the
    chip's activity throttle (~50% util limit), which also co-throttles the
    PE to mid p-state, so total bytes moved is the first-order knob.

Legacy modes (BASS_CONV_MODE): "f32r" (chunked serial layout, rel err
~2e-4, ~205 us), "bf16" (chunked paired layout, ~168 us), "f32" (~230 us).
"""

import os
import sys

for _p in ("/opt/trn_rl_repo",):
    if os.path.isdir(_p) and _p not in sys.path:
        sys.path.append(_p)

import numpy as np

# Problem dims (hardcoded per contract)
B, CIN, COUT = 32, 32, 64
H, W = 128, 128
KH = KW = 3
HO = WO = 126
MAXCN = 64
NCORES = 8
BL = B // NCORES  # local batch per core

# chunks of output rows per image: (x_row_start, n_x_rows, out_row_start, n_out_rows)
CHUNKS = [(0, 34, 0, 32), (32, 34, 32, 32), (64, 34, 64, 32), (96, 32, 96, 30)]
X3W = 34 * W  # x3 tile free size (elements)

_MODE = os.environ.get("BASS_CONV_MODE", "v2")

_RUNNER_CACHE = {}


def _split_waits(nc, mybir, maxw=1):
    """This walrus build only accepts one sem-wait per instruction; hoist
    extra waits onto preceding NoOps on the same engine."""
    for f in nc.m.functions:
        for bb in f.blocks:
            newlist = []
            for inst in bb.instructions:
                si = inst.sync_info
                waits = list(si.on_wait) if si and si.on_wait else []
                if len(waits) > maxw:
                    chunks = [waits[i : i + maxw] for i in range(0, len(waits), maxw)]
                    for ci, ch in enumerate(chunks[:-1]):
                        nop = mybir.InstNoOp(
                            name=f"{inst.name}-ws{ci}", ins=[], outs=[]
                        )
                        nop.engine = inst.engine
                        nop.sync_info = mybir.SyncInfo(on_wait=list(ch), on_update=[])
                        newlist.append(nop)
                    si.on_wait = chunks[-1]
                newlist.append(inst)
            bb.instructions = newlist


def build_nc_v2(split_waits=True):
    """v2 layout: (y, y+64) psum half-pairing.

    vs the chunked baseline:
      - bias is read from HBM exactly once (lower rows -> partitions 0:64,
        upper rows -> 64:128 at the same free offset), so the single
        [128, N] psum+bias vector add needs no shifted second copy.
      - x is loaded whole-image (one f32->bf16 cast DMA per image) and the
        two row-shifted partition blocks are built by ACT / DVE copies, not
        DMA, taking ~12.5 MB/core off the DMA engines (the bottleneck).
      - stores are 16-contiguous-output-row blocks (32 KB per channel), so
        trigger/descriptor overhead drops ~4x.
    """
    import concourse.bass as bass
    import concourse.mybir as mybir
    from concourse.tile import TileContext

    f32 = mybir.dt.float32
    i32 = mybir.dt.int32
    bf16 = mybir.dt.bfloat16

    nc = bass.Bass()
    xs = nc.declare_dram_parameter("xs", [BL, CIN, H, W], f32, isOutput=False)
    # host-permuted weights: wT2[dy*32+c, ((h*3+dx)*64+o)] = w[o, 32h+c, dy, dx]
    wt2 = nc.declare_dram_parameter("wt2", [96, 2 * 3 * COUT], f32, isOutput=False)
    bias = nc.declare_dram_parameter("bias", [COUT, HO * WO], f32, isOutput=False)
    # cn broadcast [96, (dx, o)] and per-partition thresholds c+32h [96, 2]
    cnbt = nc.declare_dram_parameter("cnbt", [96, 3 * COUT], f32, isOutput=False)
    cvals = nc.declare_dram_parameter("cvals", [96, 2], f32, isOutput=False)
    # output stored as bf16 (halves store traffic; host upcasts after gather).
    # Quantization adds <=2e-3 rel err on top of the bf16 conv's 2.6e-3.
    out = nc.declare_dram_parameter("out", [BL, COUT, HO * WO], bf16, isOutput=True)

    NG = 4  # pair-groups per image; group g: A rows [16g,16g+16), B +64
    with TileContext(nc, pool_alloc_mode="queue") as tc:
        with tc.tile_pool(name="const", bufs=1) as constp:
            bias2 = constp.tile([128, 64 * WO], f32)
            lhsT = constp.tile([96, 3 * COUT], bf16)

            # ---- main loop ----
            with (
                tc.tile_pool(name="xi", bufs=3) as xip,
                tc.tile_pool(name="ps", bufs=4, space="PSUM") as psp,
                tc.tile_pool(name="ob", bufs=2) as obp,
            ):
                imgs = {}
                pending_shifts = []

                # x row-chunks in exact group-consumption order: group g's
                # pieces need A rows up to 16g+18 and B rows up to 80+16g+2,
                # so alternating A/B eighths minimize each piece's queue wait
                QA = [(0, 18), (64, 82), (18, 34), (82, 98)]
                QB = [(34, 50), (98, 114), (50, 66), (114, 128)]

                def issue_img_a(bi, halves=False):
                    xi = xip.tile([96, H * W], bf16, tag="xi", name=f"xi_{bi}")
                    imgs[bi] = xi
                    # f32 -> bf16 cast DMA must be gpsimd (software DGE)
                    for r0, r1 in QA:
                        nc.gpsimd.dma_start(
                            out=xi[0:32, r0 * W : r1 * W], in_=xs[bi, :, r0:r1, :]
                        )
                    pending_shifts.extend((bi, g) for g in range(NG))
                    pending_act.extend((bi, g) for g in range(NG))

                def issue_img_b(bi, halves=False):
                    xi = imgs[bi]
                    for r0, r1 in QB:
                        nc.gpsimd.dma_start(
                            out=xi[0:32, r0 * W : r1 * W], in_=xs[bi, :, r0:r1, :]
                        )

                def shift_piece(bi, g, dy):
                    # row-shifted partition block copies for group g of image
                    # bi: dy=1 on ACT, dy=2 on DVE. ACT pieces are issued 4
                    # groups ahead (ACT is otherwise idle, so waiting on a
                    # late x-load there is harmless); DVE pieces only 2
                    # groups ahead so they never head-of-line-block the
                    # psum+bias adds behind them in the in-order DVE queue.
                    xi = imgs[bi]
                    base = 32 * dy
                    for r0 in (16 * g, 64 + 16 * g):
                        r1 = min(r0 + 16, H - dy)
                        # bitcast the bf16 copies to f32: engines move one
                        # element per lane-cycle regardless of width, so the
                        # wider view halves the copy time (bit-identical);
                        # all offsets/lengths are multiples of W=128, so the
                        # 2-per-4-byte alignment always holds
                        o = xi[base : base + 32, r0 * W : r1 * W].bitcast(f32)
                        i = xi[0:32, (r0 + dy) * W : (r1 + dy) * W].bitcast(
                            f32
                        )
                        if dy == 1:
                            nc.scalar.copy(out=o, in_=i)
                        else:
                            nc.vector.tensor_copy(out=o, in_=i)

                pending_act = []

                def pop_shift():
                    if pending_act:
                        shift_piece(*pending_act.pop(0), 1)
                    if pending_shifts:
                        shift_piece(*pending_shifts.pop(0), 2)

                def issue_bias(g):
                    a, b = 16 * g * WO, (16 * g + 16) * WO
                    nc.sync.dma_start(out=bias2[0:64, a:b], in_=bias[:, a:b])
                    ur0 = 64 + 16 * g
                    ur1 = min(HO, ur0 + 16)
                    n = (ur1 - ur0) * WO
                    nc.sync.dma_start(
                        out=bias2[64:128, a : a + n], in_=bias[:, ur0 * WO : ur1 * WO]
                    )

                # ---- weight prep: lhsT built directly in transposed layout
                # lhsT[(dy,c),(dx,o)] = sum_h wT2[(dy,c),(h,dx,o)]*(c+32h<cn[o])
                with tc.tile_pool(name="prep", bufs=1) as prepp:
                    w_sb = prepp.tile([96, 2 * 3 * COUT], f32)
                    nc.gpsimd.dma_start(out=w_sb[:], in_=wt2[:])
                    cnb_sb = prepp.tile([96, 3 * COUT], f32)
                    nc.sync.dma_start(out=cnb_sb[:], in_=cnbt[:])
                    cv_sb = prepp.tile([96, 2], f32)
                    nc.sync.dma_start(out=cv_sb[:], in_=cvals[:])

                    NF = 3 * COUT
                    wm = prepp.tile([96, 2 * NF], f32)
                    for h in range(2):
                        mh = prepp.tile([96, NF], f32)
                        # mask: cn[o] > c + 32h
                        nc.vector.tensor_scalar(
                            out=mh[:],
                            in0=cnb_sb[:],
                            scalar1=cv_sb[:, h : h + 1],
                            scalar2=None,
                            op0=mybir.AluOpType.is_gt,
                        )
                        nc.vector.tensor_mul(
                            out=wm[:, h * NF : (h + 1) * NF],
                            in0=w_sb[:, h * NF : (h + 1) * NF],
                            in1=mh[:],
                        )
                    nc.vector.tensor_add(
                        out=lhsT[:], in0=wm[:, 0:NF], in1=wm[:, NF : 2 * NF]
                    )

                issue_img_a(0, halves=True)
                issue_bias(0)
                issue_img_b(0, halves=True)
                # phase the piece FIFOs: ACT starts 4 ahead, DVE 2 ahead
                for k in range(NG):
                    if pending_act:
                        shift_piece(*pending_act.pop(0), 1)
                    if k < 2 and pending_shifts:
                        shift_piece(*pending_shifts.pop(0), 2)

                for bi in range(BL):
                    xv = imgs[bi].rearrange("p (r c) -> p r c", c=W)
                    for g in range(NG):
                        # bias pieces staggered one group ahead of their adds
                        if bi == 0 and g < NG - 1:
                            issue_bias(g + 1)
                        # staggered prefetch; xi pool holds 3 images
                        if bi == 0:
                            if g == 0:
                                issue_img_a(1)
                            elif g == 1:
                                issue_img_b(1)
                        if bi + 2 < BL:
                            if g == 2:
                                issue_img_a(bi + 2)
                            elif g == 3:
                                issue_img_b(bi + 2)
                        # two pairs share one 2-bank psum tile: pair B lives
                        # at the bank-aligned 512-element offset so one
                        # strided-AP vector add evacuates both pairs (the
                        # per-add fixed overhead dominates 504-elem adds)
                        pairs = []
                        for gi in range(4):
                            pi = 4 * g + gi
                            nrB = min(4, HO - (64 + 4 * pi))
                            if gi % 2 == 0:
                                ps = psp.tile([128, 1024], f32, tag="ps")
                            off = 512 * (gi % 2)
                            pairs.append((pi, nrB, ps, off))
                        for dx in range(3):
                            lw = lhsT[:, dx * COUT : (dx + 1) * COUT]
                            for (pi, nrB, ps, off) in pairs:
                                yA = 4 * pi
                                yB = 64 + 4 * pi
                                nc.tensor.matmul(
                                    ps[0:64, off : off + 4 * WO],
                                    lhsT=lw,
                                    rhs=xv[:, yA : yA + 4, dx : dx + WO],
                                    start=(dx == 0),
                                    stop=(dx == 2),
                                    skip_group_check=True,
                                )
                                nc.tensor.matmul(
                                    ps[64:128, off : off + nrB * WO],
                                    lhsT=lw,
                                    rhs=xv[:, yB : yB + nrB, dx : dx + WO],
                                    start=(dx == 0),
                                    stop=(dx == 2),
                                    skip_group_check=True,
                                )
                        ob = obp.tile([128, 16 * WO], bf16, tag="ob")
                        last = bi == BL - 1 and g == NG - 1
                        yA0 = 16 * g
                        yB0 = 64 + 16 * g
                        nu = min(HO, yB0 + 16) - yB0
                        for gi, (pi, nrB, ps, off) in enumerate(pairs):
                            o0 = gi * 4 * WO
                            c0 = 4 * pi * WO
                            if gi % 2 == 0:
                                continue  # evacuated by gi+1's fused add
                            if nrB == 4:
                                # fused add covering this pair and the
                                # previous one (same psum tile, k-strided)
                                pv = ps.rearrange("p (k n) -> p k n", n=512)
                                ov = ob[
                                    :, o0 - 4 * WO : o0 + 4 * WO
                                ].rearrange("p (k n) -> p k n", n=4 * WO)
                                bv = bias2[
                                    :, c0 - 4 * WO : c0 + 4 * WO
                                ].rearrange("p (k n) -> p k n", n=4 * WO)
                                nc.vector.tensor_add(
                                    out=ov, in0=pv[:, :, 0 : 4 * WO], in1=bv
                                )
                            else:
                                # ragged final pair: evacuate its partner
                                # first, then the ragged halves
                                nc.vector.tensor_add(
                                    out=ob[:, o0 - 4 * WO : o0],
                                    in0=ps[:, 0 : 4 * WO],
                                    in1=bias2[:, c0 - 4 * WO : c0],
                                )
                                nB = nrB * WO
                                nc.vector.tensor_add(
                                    out=ob[:, o0 : o0 + nB],
                                    in0=ps[:, off : off + nB],
                                    in1=bias2[:, c0 : c0 + nB],
                                )
                                nc.vector.tensor_add(
                                    out=ob[0:64, o0 + nB : o0 + 4 * WO],
                                    in0=ps[0:64, off + nB : off + 4 * WO],
                                    in1=bias2[0:64, c0 + nB : c0 + 4 * WO],
                                )
                            if last and gi == 1:
                                # drain the first half of the final group while
                                # the second half's adds are still running
                                nc.sync.dma_start(
                                    out=out[bi, :, yA0 * WO : (yA0 + 8) * WO],
                                    in_=ob[0:64, 0 : 8 * WO],
                                )
                                nc.sync.dma_start(
                                    out=out[bi, :, yB0 * WO : (yB0 + 8) * WO],
                                    in_=ob[64:128, 0 : 8 * WO],
                                )
                        if last:
                            nc.sync.dma_start(
                                out=out[bi, :, (yA0 + 8) * WO : (yA0 + 16) * WO],
                                in_=ob[0:64, 8 * WO :],
                            )
                            nc.sync.dma_start(
                                out=out[
                                    bi, :, (yB0 + 8) * WO : (yB0 + nu) * WO
                                ],
                                in_=ob[64:128, 8 * WO : nu * WO],
                            )
                        else:
                            nc.sync.dma_start(
                                out=out[bi, :, yA0 * WO : (yA0 + 16) * WO],
                                in_=ob[0:64, :],
                            )
                            nc.sync.dma_start(
                                out=out[bi, :, yB0 * WO : (yB0 + nu) * WO],
                                in_=ob[64:128, 0 : nu * WO],
                            )
                        # one shift piece per group, issued after the adds so
                        # a late x-load can't head-of-line-block them
                        pop_shift()
                        # remaining bias pieces go behind the first stores in
                        # the SP queue so startup stores aren't delayed
                        if bi == 0 and g < 2:
                            issue_bias(g + 2)

    if split_waits:
        _split_waits(nc, mybir)
    return nc


def build_nc(mode=_MODE, split_waits=True):
    if mode == "v2":
        return build_nc_v2(split_waits=split_waits)
    import concourse.bass as bass
    import concourse.mybir as mybir
    from concourse.tile import TileContext

    f32 = mybir.dt.float32
    i32 = mybir.dt.int32
    if mode == "bf16":
        mmdt = mybir.dt.bfloat16
    elif mode == "f32r":
        mmdt = mybir.dt.float32r
    else:
        mmdt = f32
    # storage dtype of matmul operand tiles: the BIR verifier requires fp32r
    # matmul operands to be *produced* as float32r, so the x3/lhsT tiles are
    # declared float32r and the copies into them perform the rounding.
    stdt = mmdt if mode in ("bf16", "f32r") else f32

    # f32r matmuls cannot target psum partitions 64:128 (ISA: dst partition
    # must be 0 for 4-byte non-exact modes), so f32r runs the "serial"
    # layout: one [64, N] psum tile at base 0 per output row-tile. bf16/f32
    # run the "paired" layout: two row-tiles concurrently via PE column
    # tiling (psum halves 0:64 / 64:128).
    paired = mode != "f32r"

    nc = bass.Bass()
    xs = nc.declare_dram_parameter("xs", [BL, CIN, H, W], f32, isOutput=False)
    wt = nc.declare_dram_parameter("wt", [COUT, MAXCN * 9], f32, isOutput=False)
    bias = nc.declare_dram_parameter("bias", [COUT, HO * WO], f32, isOutput=False)
    cn = nc.declare_dram_parameter("cn", [COUT, 1], i32, isOutput=False)
    iota = nc.declare_dram_parameter("iota", [COUT, MAXCN], f32, isOutput=False)
    ident = nc.declare_dram_parameter("ident", [COUT, COUT], f32, isOutput=False)
    out = nc.declare_dram_parameter("out", [BL, COUT, HO * WO], f32, isOutput=True)

    def as_mm(ap):
        return ap

    # queue-mode SBUF allocator: freed prep-pool space is not immediately
    # reused by the x3 pool, so x3 loads don't inherit a WAR dependency on
    # the weight-prep chain
    with TileContext(nc, pool_alloc_mode="queue") as tc:
        with tc.tile_pool(name="const", bufs=1) as constp:
            # persistent tiles
            bias2 = constp.tile([128 if paired else 64, HO * WO], f32)
            lhsT = constp.tile([96, 3 * COUT], stdt)

            # ---- weight prep ----
            with (
                tc.tile_pool(name="prep", bufs=1) as prepp,
                tc.tile_pool(name="tps", bufs=3, space="PSUM") as tpsp,
            ):
                prep_dmas = []
                w_sb = prepp.tile([COUT, MAXCN * 9], f32)
                prep_dmas.append(nc.sync.dma_start(out=w_sb[:], in_=wt[:]))
                cn_i = prepp.tile([COUT, 1], i32)
                prep_dmas.append(nc.sync.dma_start(out=cn_i[:], in_=cn[:]))
                iota_sb = prepp.tile([COUT, MAXCN], f32)
                prep_dmas.append(nc.sync.dma_start(out=iota_sb[:], in_=iota[:]))
                ident_sb = prepp.tile([COUT, COUT], f32)
                prep_dmas.append(nc.sync.dma_start(out=ident_sb[:], in_=ident[:]))

                cn_f = prepp.tile([COUT, 1], f32)
                nc.vector.tensor_copy(out=cn_f[:], in_=cn_i[:])
                mask = prepp.tile([COUT, MAXCN], f32)
                # mask[o, j] = (j < cn[o]) -> 1.0 / 0.0
                nc.vector.tensor_scalar(
                    out=mask[:],
                    in0=iota_sb[:],
                    scalar1=cn_f[:],
                    scalar2=None,
                    op0=mybir.AluOpType.is_lt,
                )
                # replicate mask over the 9 (dy,dx) slots: mask9[o, j*9+k]
                mask9 = prepp.tile([COUT, MAXCN * 9], f32)
                m9v = mask9.rearrange("p (j k) -> p j k", k=9)
                for k in range(9):
                    nc.vector.tensor_copy(out=m9v[:, :, k], in_=mask[:])
                wm = prepp.tile([COUT, MAXCN * 9], f32)
                nc.vector.tensor_mul(out=wm[:], in0=w_sb[:], in1=mask9[:])
                # fold j and j+32 (same input channel): kd[o, (c, dy, dx)]
                kd = prepp.tile([COUT, CIN * 9], f32)
                nc.vector.tensor_add(
                    out=kd[:], in0=wm[:, 0 : CIN * 9], in1=wm[:, CIN * 9 : MAXCN * 9]
                )
                # reorder to (dx, dy, c) contiguous, then transpose per dx:
                # [64, (dy, c)] -> [96, 64]
                kd4 = kd.rearrange("p (c dy dx) -> p dx dy c", c=CIN, dy=3, dx=3)
                kdr = prepp.tile([COUT, CIN * 9], f32)
                kdr4 = kdr.rearrange("p (dx dy c) -> p dx dy c", c=CIN, dy=3, dx=3)
                for dx in range(3):
                    nc.vector.tensor_copy(out=kdr4[:, dx], in_=kd4[:, dx])
                for dx in range(3):
                    tp = tpsp.tile([96, COUT], f32)
                    nc.tensor.transpose(
                        out=tp[:],
                        in_=kdr[:, dx * 96 : (dx + 1) * 96],
                        identity=ident_sb[:],
                    )
                    nc.vector.tensor_copy(
                        out=lhsT[:, dx * COUT : (dx + 1) * COUT], in_=tp[:]
                    )

            # ---- main loop (software-pipelined chunks) ----
            chunks = [
                (bi, r0, nxr, oy0, nor)
                for bi in range(BL)
                for (r0, nxr, oy0, nor) in CHUNKS
            ]
            with (
                tc.tile_pool(name="x3", bufs=4) as x3p,
                tc.tile_pool(name="ps", bufs=8 if not paired else 6, space="PSUM") as psp,
                tc.tile_pool(name="ob", bufs=2) as obp,
            ):
                x3_tiles = {}

                from concourse.tile_rust import add_dep_helper

                def issue_x3(ci):
                    bi, r0, nxr, oy0, nor = chunks[ci]
                    x3 = x3p.tile([96, X3W], stdt, tag="x3", name=f"x3_{ci}")
                    if mode == "bf16":
                        ld = nc.gpsimd.dma_start(
                            out=x3[0:32, 0 : nxr * W], in_=xs[bi, :, r0 : r0 + nxr, :]
                        )
                    else:
                        xin = xs[bi, :, r0 : r0 + nxr, :]
                        if mode == "f32r":
                            xin = xin.bitcast(mmdt)
                        ld = nc.sync.dma_start(out=x3[0:32, 0 : nxr * W], in_=xin)

                    nc.scalar.dma_start(
                        out=x3[32:64, 0 : nor * W],
                        in_=x3[0:32, W : (nor + 1) * W],
                    )
                    nc.gpsimd.dma_start(
                        out=x3[64:96, 0 : nor * W],
                        in_=x3[0:32, 2 * W : (nor + 2) * W],
                    )
                    x3_tiles[ci] = x3

                issue_x3(0)
                issue_x3(1)

                # bias is loaded piecewise during the first batch's chunks so
                # its 4 MB doesn't flood the DMA fabric at startup. Upper half
                # (paired only): shifted by 4 output rows so one [128, N]
                # vector add covers a psum pair.
                SH = 4 * WO

                def issue_bias(ci):
                    _, _, _, oy0, nor = chunks[ci]
                    a, b = oy0 * WO, (oy0 + nor) * WO
                    nc.gpsimd.dma_start(out=bias2[0:64, a:b], in_=bias[:, a:b])
                    if paired:
                        hb = min(b - SH, HO * WO - SH)
                        nc.gpsimd.dma_start(
                            out=bias2[64:128, a - SH if a >= SH else 0 : hb],
                            in_=bias[:, max(a, SH) : hb + SH],
                        )

                for ci, (bi, r0, nxr, oy0, nor) in enumerate(chunks):
                    if ci < len(CHUNKS):
                        issue_bias(ci)
                    if ci + 2 < len(chunks):
                        issue_x3(ci + 2)
                    x3 = x3_tiles.pop(ci)
                    x3v = x3.rearrange("p (r c) -> p r c", c=W)
                    if True:

                        if paired:
                            # pairs of two 4-row tiles (slots A/B via PE col
                            # tiling); dx-outer over groups of 3 pairs so the
                            # stationary weights only change every 6 matmuls
                            npairs = nor // 8
                            rem = nor - npairs * 8  # 0 or 6
                            allp = npairs + (1 if rem else 0)
                            p0 = 0
                            while p0 < allp:
                                g = min(4, allp - p0)
                                pss = []
                                for pi in range(p0, p0 + g):
                                    nr = 4 if pi < npairs else rem // 2
                                    ps = psp.tile([128, 4 * WO], f32, tag="ps")
                                    pss.append((pi, nr, ps))
                                for dx in range(3):
                                    lw = as_mm(lhsT[:, dx * COUT : (dx + 1) * COUT])
                                    for (pi, nr, ps) in pss:
                                        yl = 8 * pi
                                        N = nr * WO
                                        nc.tensor.matmul(
                                            ps[0:64, 0:N],
                                            lhsT=lw,
                                            rhs=as_mm(
                                                x3v[:, yl : yl + nr, dx : dx + WO]
                                            ),
                                            start=(dx == 0),
                                            stop=(dx == 2),
                                            skip_group_check=True,
                                        )
                                        nc.tensor.matmul(
                                            ps[64:128, 0:N],
                                            lhsT=lw,
                                            rhs=as_mm(
                                                x3v[
                                                    :,
                                                    yl + nr : yl + 2 * nr,
                                                    dx : dx + WO,
                                                ]
                                            ),
                                            start=(dx == 0),
                                            stop=(dx == 2),
                                            skip_group_check=True,
                                        )
                                ob = obp.tile([128, 4 * 4 * WO], f32, tag="ob")
                                for gi, (pi, nr, ps) in enumerate(pss):
                                    yg = oy0 + 8 * pi
                                    N = nr * WO
                                    o0 = gi * 4 * WO
                                    if nr == 4:
                                        nc.vector.tensor_add(
                                            out=ob[:, o0 : o0 + N],
                                            in0=ps[:, 0:N],
                                            in1=bias2[:, yg * WO : yg * WO + N],
                                        )
                                    else:
                                        nc.vector.tensor_add(
                                            out=ob[0:64, o0 : o0 + N],
                                            in0=ps[0:64, 0:N],
                                            in1=bias2[0:64, yg * WO : yg * WO + N],
                                        )
                                        nc.vector.tensor_add(
                                            out=ob[64:128, o0 : o0 + N],
                                            in0=ps[64:128, 0:N],
                                            in1=bias2[
                                                64:128,
                                                (yg + nr) * WO
                                                - SH : (yg + nr) * WO
                                                - SH
                                                + N,
                                            ],
                                        )
                                # one batched store per group half; slot-A
                                # rows sit at even 4-row blocks, slot-B at odd
                                fullg = all(nr == 4 for (_, nr, _) in pss)
                                yg0 = oy0 + 8 * p0
                                if fullg:
                                    dv = out[
                                        bi, :, yg0 * WO : (yg0 + 8 * g) * WO
                                    ].rearrange(
                                        "o (pb h n) -> o pb h n", h=2, n=4 * WO
                                    )
                                    obv = ob.rearrange("p (t n) -> p t n", n=4 * WO)
                                    nc.scalar.dma_start(
                                        out=dv[:, :, 0, :], in_=obv[0:64, 0:g]
                                    )
                                    nc.scalar.dma_start(
                                        out=dv[:, :, 1, :], in_=obv[64:128, 0:g]
                                    )
                                else:
                                    for gi, (pi, nr, ps) in enumerate(pss):
                                        yg = oy0 + 8 * pi
                                        N = nr * WO
                                        o0 = gi * 4 * WO
                                        nc.scalar.dma_start(
                                            out=out[bi, :, yg * WO : yg * WO + N],
                                            in_=ob[0:64, o0 : o0 + N],
                                        )
                                        nc.scalar.dma_start(
                                            out=out[
                                                bi,
                                                :,
                                                (yg + nr) * WO : (yg + nr) * WO + N,
                                            ],
                                            in_=ob[64:128, o0 : o0 + N],
                                        )
                                p0 += g
                        else:
                            # serial layout: 4-row tiles, dx-outer over groups
                            # of 6 tiles (weights change every 6 matmuls)
                            ntiles = (nor + 3) // 4
                            t0 = 0
                            while t0 < ntiles:
                                g = min(8, ntiles - t0)
                                tss = []
                                for ti in range(t0, t0 + g):
                                    nr = min(4, nor - 4 * ti)
                                    ps = psp.tile([64, 4 * WO], f32, tag="ps")
                                    tss.append((ti, nr, ps))
                                for dx in range(3):
                                    lw = lhsT[:, dx * COUT : (dx + 1) * COUT]
                                    for (ti, nr, ps) in tss:
                                        yl = 4 * ti
                                        nc.tensor.matmul(
                                            ps[:, 0 : nr * WO],
                                            lhsT=lw,
                                            rhs=x3v[:, yl : yl + nr, dx : dx + WO],
                                            start=(dx == 0),
                                            stop=(dx == 2),
                                            skip_group_check=True,
                                        )
                                ob = obp.tile([64, 8 * 4 * WO], f32, tag="ob")
                                rows = 0
                                for gi, (ti, nr, ps) in enumerate(tss):
                                    yg = oy0 + 4 * ti
                                    N = nr * WO
                                    o0 = gi * 4 * WO
                                    nc.vector.tensor_add(
                                        out=ob[:, o0 : o0 + N],
                                        in0=ps[:, 0:N],
                                        in1=bias2[0:64, yg * WO : yg * WO + N],
                                    )
                                    rows += nr
                                yg0 = oy0 + 4 * t0
                                if rows == 4 * g:
                                    nc.scalar.dma_start(
                                        out=out[
                                            bi, :, yg0 * WO : (yg0 + rows) * WO
                                        ],
                                        in_=ob[:, 0 : rows * WO],
                                    )
                                else:
                                    # ragged tail: last tile shorter; store
                                    # full tiles in one DMA, tail separately
                                    nf = rows - tss[-1][1]
                                    nc.scalar.dma_start(
                                        out=out[bi, :, yg0 * WO : (yg0 + nf) * WO],
                                        in_=ob[:, 0 : nf * WO],
                                    )
                                    nc.scalar.dma_start(
                                        out=out[
                                            bi,
                                            :,
                                            (yg0 + nf) * WO : (yg0 + rows) * WO,
                                        ],
                                        in_=ob[
                                            :,
                                            (g - 1) * 4 * WO : (g - 1) * 4 * WO
                                            + tss[-1][1] * WO,
                                        ],
                                    )
                                t0 += g

    if split_waits:
        _split_waits(nc, mybir)
    return nc


def _make_inputs(x, weights, bias, connect_nums):
    """Host-side reshapes only (no input-dependent compute)."""
    x = np.ascontiguousarray(np.asarray(x, dtype=np.float32))
    w = np.ascontiguousarray(
        np.asarray(weights, dtype=np.float32).reshape(COUT, MAXCN * 9)
    )
    b = np.ascontiguousarray(np.asarray(bias, dtype=np.float32).reshape(COUT, HO * WO))
    cnv = np.ascontiguousarray(
        np.asarray(connect_nums, dtype=np.int32).reshape(COUT, 1)
    )
    iota = np.ascontiguousarray(
        np.tile(np.arange(MAXCN, dtype=np.float32), (COUT, 1))
    )
    iota9 = np.ascontiguousarray(
        np.tile(np.repeat(np.arange(MAXCN, dtype=np.float32), 9), (COUT, 1))
    )
    ident = np.eye(COUT, dtype=np.float32)
    # v2 layout: wT2[dy*32+c, ((h*3+dx)*64+o)] = w[o, 32h+c, dy, dx]
    wr = np.asarray(weights, dtype=np.float32).reshape(COUT, 2, CIN, KH, KW)
    wt2 = np.ascontiguousarray(
        wr.transpose(3, 2, 1, 4, 0).reshape(96, 2 * 3 * COUT)
    )
    cnf = np.asarray(connect_nums, dtype=np.float32)
    cnbt = np.ascontiguousarray(np.tile(cnf[None, :], (96, 3)))
    cc = (np.arange(96) % CIN).astype(np.float32)
    cvals = np.ascontiguousarray(np.stack([cc, cc + CIN], axis=1))
    shards = x.reshape(NCORES, BL, CIN, H, W)
    in_maps = [
        {
            "xs": shards[c],
            "wt": w,
            "bias": b,
            "cn": cnv,
            "iota": iota,
            "iota9": iota9,
            "ident": ident,
            "wt2": wt2,
            "cnbt": cnbt,
            "cvals": cvals,
        }
        for c in range(NCORES)
    ]
    return in_maps


def _get_runner(mode=_MODE):
    """Build + jit once; reuse across kernel() calls."""
    if mode in _RUNNER_CACHE:
        return _RUNNER_CACHE[mode]

    import jax
    from jax.experimental.shard_map import shard_map
    from jax.sharding import Mesh, PartitionSpec

    import concourse.mybir as mybir
    from concourse.bass2jax import (
        _bass_exec_p,
        install_neuronx_cc_hook,
        partition_id_tensor,
    )

    nc = build_nc(mode)
    install_neuronx_cc_hook()

    partition_name = nc.partition_id_tensor.name if nc.partition_id_tensor else None
    in_names = []
    out_names = []
    out_avals = []
    zero_shapes = []
    for alloc in nc.m.functions[0].allocations:
        if not isinstance(alloc, mybir.MemoryLocationSet):
            continue
        name = alloc.memorylocations[0].name
        if alloc.kind == "ExternalInput":
            if name != partition_name:
                in_names.append(name)
        elif alloc.kind == "ExternalOutput":
            out_names.append(name)
            shape = tuple(alloc.tensor_shape)
            dtype = mybir.dt.np(alloc.dtype)
            out_avals.append(jax.core.ShapedArray(shape, dtype))
            zero_shapes.append((shape, dtype))
    n_params = len(in_names)
    n_outs = len(out_names)
    all_names = in_names + out_names
    if partition_name is not None:
        all_names = all_names + [partition_name]

    def _body(*args):
        operands = list(args)
        if partition_name is not None:
            operands.append(partition_id_tensor())
        outs = _bass_exec_p.bind(
            *operands,
            out_avals=tuple(out_avals),
            in_names=tuple(all_names),
            out_names=tuple(out_names),
            lowering_input_output_aliases=(),
            sim_require_finite=True,
            sim_require_nnan=True,
            nc=nc,
        )
        return tuple(outs)

    devices = jax.devices()[:NCORES]
    mesh = Mesh(np.asarray(devices), ("core",))
    in_specs = (PartitionSpec("core"),) * (n_params + n_outs)
    out_specs = (PartitionSpec("core"),) * n_outs
    sharded = jax.jit(
        shard_map(
            _body, mesh=mesh, in_specs=in_specs, out_specs=out_specs, check_rep=False
        ),
        donate_argnums=tuple(range(n_params, n_params + n_outs)),
        keep_unused=True,
    )

    def run(in_maps):
        concat_in = [
            np.concatenate([np.asarray(in_maps[c][nm]) for c in range(NCORES)], axis=0)
            for nm in in_names
        ]
        concat_zeros = [
            np.zeros((NCORES * s[0],) + tuple(s[1:]), dt) for (s, dt) in zero_shapes
        ]
        out_arrs = sharded(*concat_in, *concat_zeros)
        outv = np.asarray(out_arrs[0])  # single output "out"
        if outv.dtype != np.float32:
            outv = outv.astype(np.float32)
        return outv.reshape(NCORES, BL, COUT, HO, WO)

    _RUNNER_CACHE[mode] = run
    return run


def kernel(x, weights, bias, connect_nums):
    run = _get_runner()
    in_maps = _make_inputs(x, weights, bias, connect_nums)
    outs = run(in_maps)
    return np.ascontiguousarray(outs.reshape(B, COUT, HO, WO))



# revision 47
# speedup vs baseline: 1.0124x; 1.0124x over previous
"""Bass/Tile Trainium2 kernel for masked-bank BatchConv2D.

Math (matches the reference nn.Module):
    mask[o, j]   = j < connect_nums[o]                       (j in [0, 64))
    kdense[o, c] = sum_{j : j%32==c} weights[o, j] * mask[o, j]   -> [64, 32, 3, 3]
    out          = conv2d(x, kdense, VALID) + bias[None]          -> [B, 64, 126, 126]

Strategy: data-parallel over batch (8 cores x 4 images). Per core, conv is
computed as 3 accumulating bf16 matmuls (one per kernel-column dx) with the
contraction dim packed as (dy, c) = 96 partitions; psum output partitions
are column-pair-tiled as rows (y, y+64), so one [128, 504] psum tile holds
4 output rows from each image half.

Default mode "v2" (~105-110 us, rel err ~3.7e-3 vs the 2e-2 gate):
  - whole-image x tile [96, H*W] bf16; the raw image lands in partitions
    0:32 via f32->bf16 cast DMAs (8 row-chunks in group-consumption order),
    and the dy=1 / dy=2 row-shifted partition blocks are built by ACT / DVE
    copies so they stay off the DMA engines (the bottleneck resource).
  - the (y, y+64) pairing lets bias load from HBM exactly once (lower rows
    to partitions 0:64, upper to 64:128 at the same free offset) and keeps
    the psum+bias evacuation a single [128, N] vector add per pair.
  - output is stored as bf16 (halves store traffic; the host upcasts after
    gather) in 16-contiguous-row blocks.
  - scheduling: weight prep is 5 DVE ops on host-transposed weights (no PE
    transpose); x loads are prefetched 1.5 images ahead (xi pool holds 3);
    shift copies are issued 4 groups ahead on ACT but only 2 on DVE so they
    never head-of-line-block the adds; bias is staggered one group ahead of
    its consumers. The kernel is limited by DMA engine throughput under the
    chip's activity throttle (~50% util limit), which also co-throttles the
    PE to mid p-state, so total bytes moved is the first-order knob.

Legacy modes (BASS_CONV_MODE): "f32r" (chunked serial layout, rel err
~2e-4, ~205 us), "bf16" (chunked paired layout, ~168 us), "f32" (~230 us).
"""

import os
import sys

for _p in ("/opt/trn_rl_repo",):
    if os.path.isdir(_p) and _p not in sys.path:
        sys.path.append(_p)

import numpy as np

# Problem dims (hardcoded per contract)
B, CIN, COUT = 32, 32, 64
H, W = 128, 128
KH = KW = 3
HO = WO = 126
MAXCN = 64
NCORES = 8
BL = B // NCORES  # local batch per core

# chunks of output rows per image: (x_row_start, n_x_rows, out_row_start, n_out_rows)
CHUNKS = [(0, 34, 0, 32), (32, 34, 32, 32), (64, 34, 64, 32), (96, 32, 96, 30)]
X3W = 34 * W  # x3 tile free size (elements)

_MODE = os.environ.get("BASS_CONV_MODE", "v2")

_RUNNER_CACHE = {}


def _split_waits(nc, mybir, maxw=1):
    """This walrus build only accepts one sem-wait per instruction; hoist
    extra waits onto preceding NoOps on the same engine."""
    for f in nc.m.functions:
        for bb in f.blocks:
            newlist = []
            for inst in bb.instructions:
                si = inst.sync_info
                waits = list(si.on_wait) if si and si.on_wait else []
                if len(waits) > maxw:
                    chunks = [waits[i : i + maxw] for i in range(0, len(waits), maxw)]
                    for ci, ch in enumerate(chunks[:-1]):
                        nop = mybir.InstNoOp(
                            name=f"{inst.name}-ws{ci}", ins=[], outs=[]
                        )
                        nop.engine = inst.engine
                        nop.sync_info = mybir.SyncInfo(on_wait=list(ch), on_update=[])
                        newlist.append(nop)
                    si.on_wait = chunks[-1]
                newlist.append(inst)
            bb.instructions = newlist


def build_nc_v2(split_waits=True):
    """v2 layout: (y, y+64) psum half-pairing.

    vs the chunked baseline:
      - bias is read from HBM exactly once (lower rows -> partitions 0:64,
        upper rows -> 64:128 at the same free offset), so the single
        [128, N] psum+bias vector add needs no shifted second copy.
      - x is loaded whole-image (one f32->bf16 cast DMA per image) and the
        two row-shifted partition blocks are built by ACT / DVE copies, not
        DMA, taking ~12.5 MB/core off the DMA engines (the bottleneck).
      - stores are 16-contiguous-output-row blocks (32 KB per channel), so
        trigger/descriptor overhead drops ~4x.
    """
    import concourse.bass as bass
    import concourse.mybir as mybir
    from concourse.tile import TileContext

    f32 = mybir.dt.float32
    i32 = mybir.dt.int32
    bf16 = mybir.dt.bfloat16

    nc = bass.Bass()
    xs = nc.declare_dram_parameter("xs", [BL, CIN, H, W], f32, isOutput=False)
    # host-permuted weights: wT2[dy*32+c, ((h*3+dx)*64+o)] = w[o, 32h+c, dy, dx]
    wt2 = nc.declare_dram_parameter("wt2", [96, 2 * 3 * COUT], f32, isOutput=False)
    bias = nc.declare_dram_parameter("bias", [COUT, HO * WO], f32, isOutput=False)
    # cn broadcast [96, (dx, o)] and per-partition thresholds c+32h [96, 2]
    cnbt = nc.declare_dram_parameter("cnbt", [96, 3 * COUT], f32, isOutput=False)
    cvals = nc.declare_dram_parameter("cvals", [96, 2], f32, isOutput=False)
    # output stored as bf16 (halves store traffic; host upcasts after gather).
    # Quantization adds <=2e-3 rel err on top of the bf16 conv's 2.6e-3.
    out = nc.declare_dram_parameter("out", [BL, COUT, HO * WO], bf16, isOutput=True)

    NG = 4  # pair-groups per image; group g: A rows [16g,16g+16), B +64
    with TileContext(nc, pool_alloc_mode="queue") as tc:
        with tc.tile_pool(name="const", bufs=1) as constp:
            bias2 = constp.tile([128, 64 * WO], f32)
            lhsT = constp.tile([96, 3 * COUT], bf16)

            # ---- main loop ----
            with (
                tc.tile_pool(name="xi", bufs=3) as xip,
                tc.tile_pool(name="ps", bufs=4, space="PSUM") as psp,
                tc.tile_pool(name="ob", bufs=2) as obp,
            ):
                imgs = {}
                pending_shifts = []

                # x row-chunks in exact group-consumption order: group g's
                # pieces need A rows up to 16g+18 and B rows up to 80+16g+2,
                # so alternating A/B eighths minimize each piece's queue wait
                QA = [(0, 18), (64, 82), (18, 34), (82, 98)]
                QB = [(34, 50), (98, 114), (50, 66), (114, 128)]

                def issue_img_a(bi, halves=False):
                    xi = xip.tile([96, H * W], bf16, tag="xi", name=f"xi_{bi}")
                    imgs[bi] = xi
                    # f32 -> bf16 cast DMA must be gpsimd (software DGE)
                    for r0, r1 in QA:
                        nc.gpsimd.dma_start(
                            out=xi[0:32, r0 * W : r1 * W], in_=xs[bi, :, r0:r1, :]
                        )
                    pending_shifts.extend((bi, g) for g in range(NG))
                    pending_act.extend((bi, g) for g in range(NG))

                def issue_img_b(bi, halves=False):
                    xi = imgs[bi]
                    for r0, r1 in QB:
                        nc.gpsimd.dma_start(
                            out=xi[0:32, r0 * W : r1 * W], in_=xs[bi, :, r0:r1, :]
                        )

                def shift_piece(bi, g, dy):
                    # row-shifted partition block copies for group g of image
                    # bi: dy=1 on ACT, dy=2 on DVE. ACT pieces are issued 4
                    # groups ahead (ACT is otherwise idle, so waiting on a
                    # late x-load there is harmless); DVE pieces only 2
                    # groups ahead so they never head-of-line-block the
                    # psum+bias adds behind them in the in-order DVE queue.
                    xi = imgs[bi]
                    # ACT (dy_sel==1) carries dy1 A+B and dy2 A; DVE only the
                    # dy2 B copy, keeping it clear for the psum+bias adds.
                    jobs = (
                        [(1, 16 * g), (1, 64 + 16 * g), (2, 16 * g)]
                        if dy == 1
                        else [(2, 64 + 16 * g)]
                    )
                    for dyy, r0 in jobs:
                        base = 32 * dyy
                        r1 = min(r0 + 16, H - dyy)
                        # bitcast the bf16 copies to f32: engines move one
                        # element per lane-cycle regardless of width, so the
                        # wider view halves the copy time (bit-identical);
                        # all offsets/lengths are multiples of W=128, so the
                        # 2-per-4-byte alignment always holds
                        o = xi[base : base + 32, r0 * W : r1 * W].bitcast(f32)
                        i = xi[0:32, (r0 + dyy) * W : (r1 + dyy) * W].bitcast(
                            f32
                        )
                        if dy == 1:
                            nc.scalar.copy(out=o, in_=i)
                        else:
                            nc.vector.tensor_copy(out=o, in_=i)

                pending_act = []

                def pop_shift():
                    if pending_act:
                        shift_piece(*pending_act.pop(0), 1)
                    if pending_shifts:
                        shift_piece(*pending_shifts.pop(0), 2)

                def issue_bias(g):
                    a, b = 16 * g * WO, (16 * g + 16) * WO
                    nc.sync.dma_start(out=bias2[0:64, a:b], in_=bias[:, a:b])
                    ur0 = 64 + 16 * g
                    ur1 = min(HO, ur0 + 16)
                    n = (ur1 - ur0) * WO
                    nc.sync.dma_start(
                        out=bias2[64:128, a : a + n], in_=bias[:, ur0 * WO : ur1 * WO]
                    )

                # ---- weight prep: lhsT built directly in transposed layout
                # lhsT[(dy,c),(dx,o)] = sum_h wT2[(dy,c),(h,dx,o)]*(c+32h<cn[o])
                with tc.tile_pool(name="prep", bufs=1) as prepp:
                    w_sb = prepp.tile([96, 2 * 3 * COUT], f32)
                    nc.gpsimd.dma_start(out=w_sb[:], in_=wt2[:])
                    cnb_sb = prepp.tile([96, 3 * COUT], f32)
                    nc.sync.dma_start(out=cnb_sb[:], in_=cnbt[:])
                    cv_sb = prepp.tile([96, 2], f32)
                    nc.sync.dma_start(out=cv_sb[:], in_=cvals[:])

                    NF = 3 * COUT
                    wm = prepp.tile([96, 2 * NF], f32)
                    for h in range(2):
                        mh = prepp.tile([96, NF], f32)
                        # mask: cn[o] > c + 32h
                        nc.vector.tensor_scalar(
                            out=mh[:],
                            in0=cnb_sb[:],
                            scalar1=cv_sb[:, h : h + 1],
                            scalar2=None,
                            op0=mybir.AluOpType.is_gt,
                        )
                        nc.vector.tensor_mul(
                            out=wm[:, h * NF : (h + 1) * NF],
                            in0=w_sb[:, h * NF : (h + 1) * NF],
                            in1=mh[:],
                        )
                    nc.vector.tensor_add(
                        out=lhsT[:], in0=wm[:, 0:NF], in1=wm[:, NF : 2 * NF]
                    )

                issue_img_a(0, halves=True)
                issue_bias(0)
                issue_img_b(0, halves=True)
                # phase the piece FIFOs: ACT starts 4 ahead, DVE 2 ahead
                for k in range(NG):
                    if pending_act:
                        shift_piece(*pending_act.pop(0), 1)
                    if k < 2 and pending_shifts:
                        shift_piece(*pending_shifts.pop(0), 2)

                for bi in range(BL):
                    xv = imgs[bi].rearrange("p (r c) -> p r c", c=W)
                    for g in range(NG):
                        # bias pieces staggered one group ahead of their adds
                        if bi == 0 and g < NG - 1:
                            issue_bias(g + 1)
                        # staggered prefetch; xi pool holds 3 images
                        if bi == 0:
                            if g == 0:
                                issue_img_a(1)
                            elif g == 1:
                                issue_img_b(1)
                        if bi + 2 < BL:
                            if g == 2:
                                issue_img_a(bi + 2)
                            elif g == 3:
                                issue_img_b(bi + 2)
                        # two pairs share one 2-bank psum tile: pair B lives
                        # at the bank-aligned 512-element offset so one
                        # strided-AP vector add evacuates both pairs (the
                        # per-add fixed overhead dominates 504-elem adds)
                        pairs = []
                        for gi in range(4):
                            pi = 4 * g + gi
                            nrB = min(4, HO - (64 + 4 * pi))
                            if gi % 2 == 0:
                                ps = psp.tile([128, 1024], f32, tag="ps")
                            off = 512 * (gi % 2)
                            pairs.append((pi, nrB, ps, off))
                        for dx in range(3):
                            lw = lhsT[:, dx * COUT : (dx + 1) * COUT]
                            for (pi, nrB, ps, off) in pairs:
                                yA = 4 * pi
                                yB = 64 + 4 * pi
                                nc.tensor.matmul(
                                    ps[0:64, off : off + 4 * WO],
                                    lhsT=lw,
                                    rhs=xv[:, yA : yA + 4, dx : dx + WO],
                                    start=(dx == 0),
                                    stop=(dx == 2),
                                    skip_group_check=True,
                                )
                                nc.tensor.matmul(
                                    ps[64:128, off : off + nrB * WO],
                                    lhsT=lw,
                                    rhs=xv[:, yB : yB + nrB, dx : dx + WO],
                                    start=(dx == 0),
                                    stop=(dx == 2),
                                    skip_group_check=True,
                                )
                        ob = obp.tile([128, 16 * WO], bf16, tag="ob")
                        last = bi == BL - 1 and g == NG - 1
                        yA0 = 16 * g
                        yB0 = 64 + 16 * g
                        nu = min(HO, yB0 + 16) - yB0
                        for gi, (pi, nrB, ps, off) in enumerate(pairs):
                            o0 = gi * 4 * WO
                            c0 = 4 * pi * WO
                            if gi % 2 == 0:
                                continue  # evacuated by gi+1's fused add
                            if nrB == 4:
                                # fused add covering this pair and the
                                # previous one (same psum tile, k-strided)
                                pv = ps.rearrange("p (k n) -> p k n", n=512)
                                ov = ob[
                                    :, o0 - 4 * WO : o0 + 4 * WO
                                ].rearrange("p (k n) -> p k n", n=4 * WO)
                                bv = bias2[
                                    :, c0 - 4 * WO : c0 + 4 * WO
                                ].rearrange("p (k n) -> p k n", n=4 * WO)
                                nc.vector.tensor_add(
                                    out=ov, in0=pv[:, :, 0 : 4 * WO], in1=bv
                                )
                            else:
                                # ragged final pair: evacuate its partner
                                # first, then the ragged halves
                                nc.vector.tensor_add(
                                    out=ob[:, o0 - 4 * WO : o0],
                                    in0=ps[:, 0 : 4 * WO],
                                    in1=bias2[:, c0 - 4 * WO : c0],
                                )
                                nB = nrB * WO
                                nc.vector.tensor_add(
                                    out=ob[:, o0 : o0 + nB],
                                    in0=ps[:, off : off + nB],
                                    in1=bias2[:, c0 : c0 + nB],
                                )
                                nc.vector.tensor_add(
                                    out=ob[0:64, o0 + nB : o0 + 4 * WO],
                                    in0=ps[0:64, off + nB : off + 4 * WO],
                                    in1=bias2[0:64, c0 + nB : c0 + 4 * WO],
                                )
                            if last and gi == 1:
                                # drain the first half of the final group while
                                # the second half's adds are still running
                                nc.sync.dma_start(
                                    out=out[bi, :, yA0 * WO : (yA0 + 8) * WO],
                                    in_=ob[0:64, 0 : 8 * WO],
                                )
                                nc.sync.dma_start(
                                    out=out[bi, :, yB0 * WO : (yB0 + 8) * WO],
                                    in_=ob[64:128, 0 : 8 * WO],
                                )
                        if last:
                            nc.sync.dma_start(
                                out=out[bi, :, (yA0 + 8) * WO : (yA0 + 16) * WO],
                                in_=ob[0:64, 8 * WO :],
                            )
                            nc.sync.dma_start(
                                out=out[
                                    bi, :, (yB0 + 8) * WO : (yB0 + nu) * WO
                                ],
                                in_=ob[64:128, 8 * WO : nu * WO],
                            )
                        else:
                            nc.sync.dma_start(
                                out=out[bi, :, yA0 * WO : (yA0 + 16) * WO],
                                in_=ob[0:64, :],
                            )
                            nc.sync.dma_start(
                                out=out[bi, :, yB0 * WO : (yB0 + nu) * WO],
                                in_=ob[64:128, 0 : nu * WO],
                            )
                        # one shift piece per group, issued after the adds so
                        # a late x-load can't head-of-line-block them
                        pop_shift()
                        # remaining bias pieces go behind the first stores in
                        # the SP queue so startup stores aren't delayed
                        if bi == 0 and g < 2:
                            issue_bias(g + 2)

    if split_waits:
        _split_waits(nc, mybir)
    return nc


def build_nc(mode=_MODE, split_waits=True):
    if mode == "v2":
        return build_nc_v2(split_waits=split_waits)
    import concourse.bass as bass
    import concourse.mybir as mybir
    from concourse.tile import TileContext

    f32 = mybir.dt.float32
    i32 = mybir.dt.int32
    if mode == "bf16":
        mmdt = mybir.dt.bfloat16
    elif mode == "f32r":
        mmdt = mybir.dt.float32r
    else:
        mmdt = f32
    # storage dtype of matmul operand tiles: the BIR verifier requires fp32r
    # matmul operands to be *produced* as float32r, so the x3/lhsT tiles are
    # declared float32r and the copies into them perform the rounding.
    stdt = mmdt if mode in ("bf16", "f32r") else f32

    # f32r matmuls cannot target psum partitions 64:128 (ISA: dst partition
    # must be 0 for 4-byte non-exact modes), so f32r runs the "serial"
    # layout: one [64, N] psum tile at base 0 per output row-tile. bf16/f32
    # run the "paired" layout: two row-tiles concurrently via PE column
    # tiling (psum halves 0:64 / 64:128).
    paired = mode != "f32r"

    nc = bass.Bass()
    xs = nc.declare_dram_parameter("xs", [BL, CIN, H, W], f32, isOutput=False)
    wt = nc.declare_dram_parameter("wt", [COUT, MAXCN * 9], f32, isOutput=False)
    bias = nc.declare_dram_parameter("bias", [COUT, HO * WO], f32, isOutput=False)
    cn = nc.declare_dram_parameter("cn", [COUT, 1], i32, isOutput=False)
    iota = nc.declare_dram_parameter("iota", [COUT, MAXCN], f32, isOutput=False)
    ident = nc.declare_dram_parameter("ident", [COUT, COUT], f32, isOutput=False)
    out = nc.declare_dram_parameter("out", [BL, COUT, HO * WO], f32, isOutput=True)

    def as_mm(ap):
        return ap

    # queue-mode SBUF allocator: freed prep-pool space is not immediately
    # reused by the x3 pool, so x3 loads don't inherit a WAR dependency on
    # the weight-prep chain
    with TileContext(nc, pool_alloc_mode="queue") as tc:
        with tc.tile_pool(name="const", bufs=1) as constp:
            # persistent tiles
            bias2 = constp.tile([128 if paired else 64, HO * WO], f32)
            lhsT = constp.tile([96, 3 * COUT], stdt)

            # ---- weight prep ----
            with (
                tc.tile_pool(name="prep", bufs=1) as prepp,
                tc.tile_pool(name="tps", bufs=3, space="PSUM") as tpsp,
            ):
                prep_dmas = []
                w_sb = prepp.tile([COUT, MAXCN * 9], f32)
                prep_dmas.append(nc.sync.dma_start(out=w_sb[:], in_=wt[:]))
                cn_i = prepp.tile([COUT, 1], i32)
                prep_dmas.append(nc.sync.dma_start(out=cn_i[:], in_=cn[:]))
                iota_sb = prepp.tile([COUT, MAXCN], f32)
                prep_dmas.append(nc.sync.dma_start(out=iota_sb[:], in_=iota[:]))
                ident_sb = prepp.tile([COUT, COUT], f32)
                prep_dmas.append(nc.sync.dma_start(out=ident_sb[:], in_=ident[:]))

                cn_f = prepp.tile([COUT, 1], f32)
                nc.vector.tensor_copy(out=cn_f[:], in_=cn_i[:])
                mask = prepp.tile([COUT, MAXCN], f32)
                # mask[o, j] = (j < cn[o]) -> 1.0 / 0.0
                nc.vector.tensor_scalar(
                    out=mask[:],
                    in0=iota_sb[:],
                    scalar1=cn_f[:],
                    scalar2=None,
                    op0=mybir.AluOpType.is_lt,
                )
                # replicate mask over the 9 (dy,dx) slots: mask9[o, j*9+k]
                mask9 = prepp.tile([COUT, MAXCN * 9], f32)
                m9v = mask9.rearrange("p (j k) -> p j k", k=9)
                for k in range(9):
                    nc.vector.tensor_copy(out=m9v[:, :, k], in_=mask[:])
                wm = prepp.tile([COUT, MAXCN * 9], f32)
                nc.vector.tensor_mul(out=wm[:], in0=w_sb[:], in1=mask9[:])
                # fold j and j+32 (same input channel): kd[o, (c, dy, dx)]
                kd = prepp.tile([COUT, CIN * 9], f32)
                nc.vector.tensor_add(
                    out=kd[:], in0=wm[:, 0 : CIN * 9], in1=wm[:, CIN * 9 : MAXCN * 9]
                )
                # reorder to (dx, dy, c) contiguous, then transpose per dx:
                # [64, (dy, c)] -> [96, 64]
                kd4 = kd.rearrange("p (c dy dx) -> p dx dy c", c=CIN, dy=3, dx=3)
                kdr = prepp.tile([COUT, CIN * 9], f32)
                kdr4 = kdr.rearrange("p (dx dy c) -> p dx dy c", c=CIN, dy=3, dx=3)
                for dx in range(3):
                    nc.vector.tensor_copy(out=kdr4[:, dx], in_=kd4[:, dx])
                for dx in range(3):
                    tp = tpsp.tile([96, COUT], f32)
                    nc.tensor.transpose(
                        out=tp[:],
                        in_=kdr[:, dx * 96 : (dx + 1) * 96],
                        identity=ident_sb[:],
                    )
                    nc.vector.tensor_copy(
                        out=lhsT[:, dx * COUT : (dx + 1) * COUT], in_=tp[:]
                    )

            # ---- main loop (software-pipelined chunks) ----
            chunks = [
                (bi, r0, nxr, oy0, nor)
                for bi in range(BL)
                for (r0, nxr, oy0, nor) in CHUNKS
            ]
            with (
                tc.tile_pool(name="x3", bufs=4) as x3p,
                tc.tile_pool(name="ps", bufs=8 if not paired else 6, space="PSUM") as psp,
                tc.tile_pool(name="ob", bufs=2) as obp,
            ):
                x3_tiles = {}

                from concourse.tile_rust import add_dep_helper

                def issue_x3(ci):
                    bi, r0, nxr, oy0, nor = chunks[ci]
                    x3 = x3p.tile([96, X3W], stdt, tag="x3", name=f"x3_{ci}")
                    if mode == "bf16":
                        ld = nc.gpsimd.dma_start(
                            out=x3[0:32, 0 : nxr * W], in_=xs[bi, :, r0 : r0 + nxr, :]
                        )
                    else:
                        xin = xs[bi, :, r0 : r0 + nxr, :]
                        if mode == "f32r":
                            xin = xin.bitcast(mmdt)
                        ld = nc.sync.dma_start(out=x3[0:32, 0 : nxr * W], in_=xin)

                    nc.scalar.dma_start(
                        out=x3[32:64, 0 : nor * W],
                        in_=x3[0:32, W : (nor + 1) * W],
                    )
                    nc.gpsimd.dma_start(
                        out=x3[64:96, 0 : nor * W],
                        in_=x3[0:32, 2 * W : (nor + 2) * W],
                    )
                    x3_tiles[ci] = x3

                issue_x3(0)
                issue_x3(1)

                # bias is loaded piecewise during the first batch's chunks so
                # its 4 MB doesn't flood the DMA fabric at startup. Upper half
                # (paired only): shifted by 4 output rows so one [128, N]
                # vector add covers a psum pair.
                SH = 4 * WO

                def issue_bias(ci):
                    _, _, _, oy0, nor = chunks[ci]
                    a, b = oy0 * WO, (oy0 + nor) * WO
                    nc.gpsimd.dma_start(out=bias2[0:64, a:b], in_=bias[:, a:b])
                    if paired:
                        hb = min(b - SH, HO * WO - SH)
                        nc.gpsimd.dma_start(
                            out=bias2[64:128, a - SH if a >= SH else 0 : hb],
                            in_=bias[:, max(a, SH) : hb + SH],
                        )

                for ci, (bi, r0, nxr, oy0, nor) in enumerate(chunks):
                    if ci < len(CHUNKS):
                        issue_bias(ci)
                    if ci + 2 < len(chunks):
                        issue_x3(ci + 2)
                    x3 = x3_tiles.pop(ci)
                    x3v = x3.rearrange("p (r c) -> p r c", c=W)
                    if True:

                        if paired:
                            # pairs of two 4-row tiles (slots A/B via PE col
                            # tiling); dx-outer over groups of 3 pairs so the
                            # stationary weights only change every 6 matmuls
                            npairs = nor // 8
                            rem = nor - npairs * 8  # 0 or 6
                            allp = npairs + (1 if rem else 0)
                            p0 = 0
                            while p0 < allp:
                                g = min(4, allp - p0)
                                pss = []
                                for pi in range(p0, p0 + g):
                                    nr = 4 if pi < npairs else rem // 2
                                    ps = psp.tile([128, 4 * WO], f32, tag="ps")
                                    pss.append((pi, nr, ps))
                                for dx in range(3):
                                    lw = as_mm(lhsT[:, dx * COUT : (dx + 1) * COUT])
                                    for (pi, nr, ps) in pss:
                                        yl = 8 * pi
                                        N = nr * WO
                                        nc.tensor.matmul(
                                            ps[0:64, 0:N],
                                            lhsT=lw,
                                            rhs=as_mm(
                                                x3v[:, yl : yl + nr, dx : dx + WO]
                                            ),
                                            start=(dx == 0),
                                            stop=(dx == 2),
                                            skip_group_check=True,
                                        )
                                        nc.tensor.matmul(
                                            ps[64:128, 0:N],
                                            lhsT=lw,
                                            rhs=as_mm(
                                                x3v[
                                                    :,
                                                    yl + nr : yl + 2 * nr,
                                                    dx : dx + WO,
                                                ]
                                            ),
                                            start=(dx == 0),
                                            stop=(dx == 2),
                                            skip_group_check=True,
                                        )
                                ob = obp.tile([128, 4 * 4 * WO], f32, tag="ob")
                                for gi, (pi, nr, ps) in enumerate(pss):
                                    yg = oy0 + 8 * pi
                                    N = nr * WO
                                    o0 = gi * 4 * WO
                                    if nr == 4:
                                        nc.vector.tensor_add(
                                            out=ob[:, o0 : o0 + N],
                                            in0=ps[:, 0:N],
                                            in1=bias2[:, yg * WO : yg * WO + N],
                                        )
                                    else:
                                        nc.vector.tensor_add(
                                            out=ob[0:64, o0 : o0 + N],
                                            in0=ps[0:64, 0:N],
                                            in1=bias2[0:64, yg * WO : yg * WO + N],
                                        )
                                        nc.vector.tensor_add(
                                            out=ob[64:128, o0 : o0 + N],
                                            in0=ps[64:128, 0:N],
                                            in1=bias2[
                                                64:128,
                                                (yg + nr) * WO
                                                - SH : (yg + nr) * WO
                                                - SH
                                                + N,
                                            ],
                                        )
                                # one batched store per group half; slot-A
                                # rows sit at even 4-row blocks, slot-B at odd
                                fullg = all(nr == 4 for (_, nr, _) in pss)
                                yg0 = oy0 + 8 * p0
                                if fullg:
                                    dv = out[
                                        bi, :, yg0 * WO : (yg0 + 8 * g) * WO
                                    ].rearrange(
                                        "o (pb h n) -> o pb h n", h=2, n=4 * WO
                                    )
                                    obv = ob.rearrange("p (t n) -> p t n", n=4 * WO)
                                    nc.scalar.dma_start(
                                        out=dv[:, :, 0, :], in_=obv[0:64, 0:g]
                                    )
                                    nc.scalar.dma_start(
                                        out=dv[:, :, 1, :], in_=obv[64:128, 0:g]
                                    )
                                else:
                                    for gi, (pi, nr, ps) in enumerate(pss):
                                        yg = oy0 + 8 * pi
                                        N = nr * WO
                                        o0 = gi * 4 * WO
                                        nc.scalar.dma_start(
                                            out=out[bi, :, yg * WO : yg * WO + N],
                                            in_=ob[0:64, o0 : o0 + N],
                                        )
                                        nc.scalar.dma_start(
                                            out=out[
                                                bi,
                                                :,
                                                (yg + nr) * WO : (yg + nr) * WO + N,
                                            ],
                                            in_=ob[64:128, o0 : o0 + N],
                                        )
                                p0 += g
                        else:
                            # serial layout: 4-row tiles, dx-outer over groups
                            # of 6 tiles (weights change every 6 matmuls)
                            ntiles = (nor + 3) // 4
                            t0 = 0
                            while t0 < ntiles:
                                g = min(8, ntiles - t0)
                                tss = []
                                for ti in range(t0, t0 + g):
                                    nr = min(4, nor - 4 * ti)
                                    ps = psp.tile([64, 4 * WO], f32, tag="ps")
                                    tss.append((ti, nr, ps))
                                for dx in range(3):
                                    lw = lhsT[:, dx * COUT : (dx + 1) * COUT]
                                    for (ti, nr, ps) in tss:
                                        yl = 4 * ti
                                        nc.tensor.matmul(
                                            ps[:, 0 : nr * WO],
                                            lhsT=lw,
                                            rhs=x3v[:, yl : yl + nr, dx : dx + WO],
                                            start=(dx == 0),
                                            stop=(dx == 2),
                                            skip_group_check=True,
                                        )
                                ob = obp.tile([64, 8 * 4 * WO], f32, tag="ob")
                                rows = 0
                                for gi, (ti, nr, ps) in enumerate(tss):
                                    yg = oy0 + 4 * ti
                                    N = nr * WO
                                    o0 = gi * 4 * WO
                                    nc.vector.tensor_add(
                                        out=ob[:, o0 : o0 + N],
                                        in0=ps[:, 0:N],
                                        in1=bias2[0:64, yg * WO : yg * WO + N],
                                    )
                                    rows += nr
                                yg0 = oy0 + 4 * t0
                                if rows == 4 * g:
                                    nc.scalar.dma_start(
                                        out=out[
                                            bi, :, yg0 * WO : (yg0 + rows) * WO
                                        ],
                                        in_=ob[:, 0 : rows * WO],
                                    )
                                else:
                                    # ragged tail: last tile shorter; store
                                    # full tiles in one DMA, tail separately
                                    nf = rows - tss[-1][1]
                                    nc.scalar.dma_start(
                                        out=out[bi, :, yg0 * WO : (yg0 + nf) * WO],
                                        in_=ob[:, 0 : nf * WO],
                                    )
                                    nc.scalar.dma_start(
                                        out=out[
                                            bi,
                                            :,
                                            (yg0 + nf) * WO : (yg0 + rows) * WO,
                                        ],
                                        in_=ob[
                                            :,
                                            (g - 1) * 4 * WO : (g - 1) * 4 * WO
                                            + tss[-1][1] * WO,
                                        ],
                                    )
                                t0 += g

    if split_waits:
        _split_waits(nc, mybir)
    return nc


def _make_inputs(x, weights, bias, connect_nums):
    """Host-side reshapes only (no input-dependent compute)."""
    x = np.ascontiguousarray(np.asarray(x, dtype=np.float32))
    w = np.ascontiguousarray(
        np.asarray(weights, dtype=np.float32).reshape(COUT, MAXCN * 9)
    )
    b = np.ascontiguousarray(np.asarray(bias, dtype=np.float32).reshape(COUT, HO * WO))
    cnv = np.ascontiguousarray(
        np.asarray(connect_nums, dtype=np.int32).reshape(COUT, 1)
    )
    iota = np.ascontiguousarray(
        np.tile(np.arange(MAXCN, dtype=np.float32), (COUT, 1))
    )
    iota9 = np.ascontiguousarray(
        np.tile(np.repeat(np.arange(MAXCN, dtype=np.float32), 9), (COUT, 1))
    )
    ident = np.eye(COUT, dtype=np.float32)
    # v2 layout: wT2[dy*32+c, ((h*3+dx)*64+o)] = w[o, 32h+c, dy, dx]
    wr = np.asarray(weights, dtype=np.float32).reshape(COUT, 2, CIN, KH, KW)
    wt2 = np.ascontiguousarray(
        wr.transpose(3, 2, 1, 4, 0).reshape(96, 2 * 3 * COUT)
    )
    cnf = np.asarray(connect_nums, dtype=np.float32)
    cnbt = np.ascontiguousarray(np.tile(cnf[None, :], (96, 3)))
    cc = (np.arange(96) % CIN).astype(np.float32)
    cvals = np.ascontiguousarray(np.stack([cc, cc + CIN], axis=1))
    shards = x.reshape(NCORES, BL, CIN, H, W)
    in_maps = [
        {
            "xs": shards[c],
            "wt": w,
            "bias": b,
            "cn": cnv,
            "iota": iota,
            "iota9": iota9,
            "ident": ident,
            "wt2": wt2,
            "cnbt": cnbt,
            "cvals": cvals,
        }
        for c in range(NCORES)
    ]
    return in_maps


def _get_runner(mode=_MODE):
    """Build + jit once; reuse across kernel() calls."""
    if mode in _RUNNER_CACHE:
        return _RUNNER_CACHE[mode]

    import jax
    from jax.experimental.shard_map import shard_map
    from jax.sharding import Mesh, PartitionSpec

    import concourse.mybir as mybir
    from concourse.bass2jax import (
        _bass_exec_p,
        install_neuronx_cc_hook,
        partition_id_tensor,
    )

    nc = build_nc(mode)
    install_neuronx_cc_hook()

    partition_name = nc.partition_id_tensor.name if nc.partition_id_tensor else None
    in_names = []
    out_names = []
    out_avals = []
    zero_shapes = []
    for alloc in nc.m.functions[0].allocations:
        if not isinstance(alloc, mybir.MemoryLocationSet):
            continue
        name = alloc.memorylocations[0].name
        if alloc.kind == "ExternalInput":
            if name != partition_name:
                in_names.append(name)
        elif alloc.kind == "ExternalOutput":
            out_names.append(name)
            shape = tuple(alloc.tensor_shape)
            dtype = mybir.dt.np(alloc.dtype)
            out_avals.append(jax.core.ShapedArray(shape, dtype))
            zero_shapes.append((shape, dtype))
    n_params = len(in_names)
    n_outs = len(out_names)
    all_names = in_names + out_names
    if partition_name is not None:
        all_names = all_names + [partition_name]

    def _body(*args):
        operands = list(args)
        if partition_name is not None:
            operands.append(partition_id_tensor())
        outs = _bass_exec_p.bind(
            *operands,
            out_avals=tuple(out_avals),
            in_names=tuple(all_names),
            out_names=tuple(out_names),
            lowering_input_output_aliases=(),
            sim_require_finite=True,
            sim_require_nnan=True,
            nc=nc,
        )
        return tuple(outs)

    devices = jax.devices()[:NCORES]
    mesh = Mesh(np.asarray(devices), ("core",))
    in_specs = (PartitionSpec("core"),) * (n_params + n_outs)
    out_specs = (PartitionSpec("core"),) * n_outs
    sharded = jax.jit(
        shard_map(
            _body, mesh=mesh, in_specs=in_specs, out_specs=out_specs, check_rep=False
        ),
        donate_argnums=tuple(range(n_params, n_params + n_outs)),
        keep_unused=True,
    )

    def run(in_maps):
        concat_in = [
            np.concatenate([np.asarray(in_maps[c][nm]) for c in range(NCORES)], axis=0)
            for nm in in_names
        ]
        concat_zeros = [
            np.zeros((NCORES * s[0],) + tuple(s[1:]), dt) for (s, dt) in zero_shapes
        ]
        out_arrs = sharded(*concat_in, *concat_zeros)
        outv = np.asarray(out_arrs[0])  # single output "out"
        if outv.dtype != np.float32:
            outv = outv.astype(np.float32)
        return outv.reshape(NCORES, BL, COUT, HO, WO)

    _RUNNER_CACHE[mode] = run
    return run


def kernel(x, weights, bias, connect_nums):
    run = _get_runner()
    in_maps = _make_inputs(x, weights, bias, connect_nums)
    outs = run(in_maps)
    return np.ascontiguousarray(outs.reshape(B, COUT, HO, WO))

